# revision 1
# baseline (speedup 1.0000x reference)
"""Causal self-attention (B=4, T=2048, C=1024, H=16) on 8 TRN2 NeuronCores.

Sharding: core c -> batch b = c//2, head-group g2 = c%2 (8 heads, feature
columns j0 = g2*512 .. +512).  Each core:
  - QKV projections for its 512-wide slice (Megatron column-parallel),
  - causal attention for its 8 heads (softmax without max-subtraction:
    logits ~ N(0,1), folded 1/sqrt(hd) into Wq on host),
  - partial output projection y_half @ Wp[:, slice].T.
Host sums the two partials per batch.  No collectives.

Device dataflow (per core), fp32r matmuls throughout:
  qT/kT [j,t] and v-natural [t,j] from projections; attT = K Q^T per k-tile
  in PSUM; exp on ScalarE (PSUM->SBUF); causal staircase zeroed by gpsimd
  affine_select; AV with col-packed [v|ones] stationary => yT rows 0-63 and
  rowsum row 64 in one matmul; rowsum bcast via K=1 ones matmul; reciprocal
  (approx, ~51 ULP) on VectorE; normalize fused into the PSUM->SBUF move;
  final projection contracts head-pair tiles K=128.
"""
import numpy as np

B, T, C = 4, 2048, 1024
NC = 8
P = 128
CO = 8           # c-tiles of 128 (contraction for QKV)
QB = 512         # t_q block
NQB = T // QB    # 4
NKT = T // P     # 16 k-tiles
D = 64           # head dim
W65 = 65         # [v | ones]

_CACHE = {}

# build-time tuning knobs (sweepable)
CFG = {"pp": 4, "attp": 2, "avp": 2, "aep": 12, "bcp": 1, "lag": 4}


def _build():
    from contextlib import ExitStack
    import concourse.bass as bass
    import concourse.tile as tile
    from concourse import bacc, mybir

    F32 = mybir.dt.float32
    F32R = mybir.dt.float32r
    AF = mybir.ActivationFunctionType
    MUL = mybir.AluOpType.mult

    nc = bacc.Bacc("TRN2", target_bir_lowering=False, debug=False,
                   dynamic_dma_scratch_size=2048)
    xt = nc.dram_tensor("xt", [C, T], F32R, kind="ExternalInput").ap()
    wq = nc.dram_tensor("wq", [C, 512], F32R, kind="ExternalInput").ap()
    wk = nc.dram_tensor("wk", [C, 512], F32R, kind="ExternalInput").ap()
    wv = nc.dram_tensor("wv", [C, 512], F32R, kind="ExternalInput").ap()
    wp = nc.dram_tensor("wp", [512, C], F32R, kind="ExternalInput").ap()
    ones8 = nc.dram_tensor("ones8", [P, CO], F32R, kind="ExternalInput").ap()
    onr = nc.dram_tensor("onr", [P, D], F32R, kind="ExternalInput").ap()
    zr = nc.dram_tensor("zr", [P, P], F32R, kind="ExternalInput").ap()
    out = nc.dram_tensor("out", [T, C], F32, kind="ExternalOutput").ap()

    xt3 = xt.rearrange("(co ci) t -> ci co t", ci=P)     # [128, 8, 2048]
    wq3 = wq.rearrange("(co ci) j -> ci co j", ci=P)     # [128, 8, 512]
    wk3 = wk.rearrange("(co ci) j -> ci co j", ci=P)
    wv3 = wv.rearrange("(co ci) j -> ci co j", ci=P)
    wp3 = wp.rearrange("(go gi) m -> gi go m", gi=P)     # [128, 4, 1024]

    with tile.TileContext(nc) as tc, ExitStack() as ctx:
        persist = ctx.enter_context(tc.tile_pool(name="persist", bufs=1))
        qt = [persist.tile([P, T], F32R, tag=f"qt{g}", name=f"qt{g}") for g in range(4)]
        kt = [persist.tile([P, T], F32R, tag=f"kt{g}", name=f"kt{g}") for g in range(4)]
        # v k-tiles packed into 2 tensors of 8 k-tiles each: [128, 8, 8, 65]
        vtp = [persist.tile([P, CO, CO, W65], F32R, tag=f"vtp{i}", name=f"vtp{i}")
               for i in range(2)]
        vt = [vtp[i // CO][:, i % CO] for i in range(NKT)]
        on8 = persist.tile([P, CO], F32R, tag="on8", name="on8")
        onrt = persist.tile([P, D], F32R, tag="onr", name="onr")
        zrt = persist.tile([P, P], F32R, tag="zrt", name="zrt")
        ytp = [persist.tile([P, NQB, QB], F32R, tag=f"ytp{g}", name=f"ytp{g}")
               for g in range(4)]
        yt = [[ytp[g][:, qb] for qb in range(NQB)] for g in range(4)]

        nc.scalar.dma_start(on8[:], ones8)
        nc.scalar.dma_start(zrt[:], zr)
        nc.scalar.dma_start(onrt[:], onr)

        # ---------------- Phase 1: QKV projections ----------------
        with tc.tile_pool(name="xtp", bufs=2) as xtp, \
             tc.tile_pool(name="wqkv", bufs=1) as wpool, \
             tc.tile_pool(name="pp", bufs=CFG["pp"], space="PSUM") as pp:
            wts = {}
            for proj, wdram in (("q", wq3), ("k", wk3), ("v", wv3)):
                wts[proj] = wpool.tile([P, CO, 512], F32R, tag=f"w{proj}",
                                       name=f"w{proj}")
            xt0 = xtp.tile([P, CO, QB], F32R, tag="xt", name="xt")
            # interleave first-needed chunks on the sync queue so the first
            # projection starts after ~one chunk of wq + xt
            for co in range(CO):
                nc.sync.dma_start(wts["q"][:, co], wq3[:, co])
                nc.sync.dma_start(xt0[:, co], xt3[:, co, 0:QB])
            # later-needed weights go on the scalar HWDGE queue
            nc.scalar.dma_start(wts["k"][:], wk3)
            nc.scalar.dma_start(wts["v"][:], wv3)
            for tb in range(NQB):
                if tb == 0:
                    xt_t = xt0
                else:
                    xt_t = xtp.tile([P, CO, QB], F32R, tag="xt", name="xt")
                    nc.sync.dma_start(xt_t[:], xt3[:, :, tb * QB:(tb + 1) * QB])
                for proj in ("q", "k", "v"):
                    w_t = wts[proj]
                    if proj != "v":
                        dst = qt if proj == "q" else kt
                        if tb == 0 and proj == "q":
                            # co-outer: 4 matmuls become ready per arriving
                            # wq/xt chunk instead of waiting for all 8
                            pss = [pp.tile([P, QB], F32, tag="pp", name="pp")
                                   for _ in range(4)]
                            for co in range(CO):
                                for g in range(4):
                                    nc.tensor.matmul(
                                        pss[g][:],
                                        w_t[:, co, g * P:(g + 1) * P],
                                        xt_t[:, co, :],
                                        start=(co == 0), stop=(co == CO - 1))
                            for g in range(4):
                                nc.vector.tensor_copy(
                                    dst[g][:, 0:QB], pss[g][:])
                            continue
                        for g in range(4):
                            ps = pp.tile([P, QB], F32, tag="pp", name="pp")
                            for co in range(CO):
                                nc.tensor.matmul(
                                    ps[:], w_t[:, co, g * P:(g + 1) * P],
                                    xt_t[:, co, :],
                                    start=(co == 0), stop=(co == CO - 1))
                            nc.vector.tensor_copy(
                                dst[g][:, tb * QB:(tb + 1) * QB], ps[:])
                    else:
                        # v natural: out[t-tile, j 512]
                        for tt in range(4):
                            ki = tb * 4 + tt
                            ps = pp.tile([P, QB], F32, tag="pp", name="pp")
                            for co in range(CO):
                                nc.tensor.matmul(
                                    ps[:], xt_t[:, co, tt * P:(tt + 1) * P],
                                    w_t[:, co, :],
                                    start=(co == 0), stop=(co == CO - 1))
                            nc.vector.tensor_copy(
                                vt[ki][:, :, 0:D],
                                ps[:].rearrange("p (h d) -> p h d", d=D))
                            nc.vector.tensor_copy(
                                vt[ki][:, :, D:W65], on8[:, :, None])

        # ------------- Phase 2+3: attention + out-projection -------------
        with tc.tile_pool(name="attp", bufs=CFG["attp"], space="PSUM") as attp, \
             tc.tile_pool(name="avp", bufs=CFG["avp"], space="PSUM") as avp, \
             tc.tile_pool(name="bcp", bufs=CFG.get("bcp",1), space="PSUM") as bcp, \
             tc.tile_pool(name="wptp", bufs=1) as wptp, \
             tc.tile_pool(name="aep", bufs=CFG["aep"]) as aep, \
             tc.tile_pool(name="rsp", bufs=2) as rsp, \
             tc.tile_pool(name="rcp", bufs=2) as rcp, \
             tc.tile_pool(name="outp", bufs=4) as outp:
            pop = attp  # out-proj psum shares the attention QK pool (tag att)
            wpt = wptp.tile([P, 4, C], F32R, tag="wpt", name="wpt")
            nc.scalar.dma_start(wpt[:], wp3)
            blocks = ([(g, qb) for g in range(4) for qb in range(NQB)]
                      if not CFG.get("qb_outer") else
                      [(g, qb) for qb in range(NQB) for g in range(4)])
            if CFG.get("rot_last"):
                blocks = blocks[:12] + [(3, 1), (3, 2), (3, 3), (3, 0)]
            for g, qb in blocks:
                q0 = qb * QB
                ks = list(range(qb * 4, qb * 4 + 4)) + list(range(0, qb * 4))
                ph = avp.tile([P, QB], F32, tag="ph", name="ph")    # head 2g
                ph2 = avp.tile([P, QB], F32, tag="ph", name="ph2")  # head 2g+1
                n = len(ks)
                # software-pipelined: QK+exp for ki, AV lags one step
                pend = []  # (ki, d, ae, first, last)
                for idx, ki in enumerate(ks):
                    d = (ki - qb * 4) * P if ki >= qb * 4 else -1
                    # head pair shares one 2-bank psum tile; narrow the
                    # moving operand to [d:QB) for mid-diagonal tiles (the
                    # psum region [0:d) is never read; d=384 would drop N
                    # below fp32r's 256-wide full-rate threshold)
                    dq = d if d in (P, 2 * P) else 0
                    ap = attp.tile([P, 2 * QB], F32, tag="att", name="att")
                    for h2 in range(2):
                        rows = slice(h2 * D, h2 * D + D)
                        nc.tensor.matmul(
                            ap[:, h2 * QB + dq:(h2 + 1) * QB],
                            kt[g][rows, ki * P:(ki + 1) * P],
                            qt[g][rows, q0 + dq:q0 + QB],
                            start=True, stop=True)
                    ae = aep.tile([P, 2 * QB], F32R, tag="ae", name="ae")
                    if d > 0:
                        for h2 in range(2):
                            o = h2 * QB
                            nc.scalar.activation(
                                ae[:, o + d:o + QB], ap[:, o + d:o + QB],
                                AF.Exp)
                            nc.gpsimd.affine_select(
                                out=ae[:, o + d:o + d + P],
                                in_=ae[:, o + d:o + d + P],
                                compare_op=mybir.AluOpType.is_ge,
                                fill=0.0, base=0,
                                pattern=[[1, P]], channel_multiplier=-1)
                            if d == QB - P:
                                # zero [QB-2P:QB-P) so the AV rhs can be
                                # N=256 (fp32r full rate needs N>=256)
                                nc.vector.tensor_copy(
                                    ae[:, o + QB - 2 * P:o + QB - P], zrt[:])
                    else:
                        # d == 0 diag and off-diag: one contiguous flat exp
                        nc.scalar.activation(ae[:], ap[:], AF.Exp)
                        if d == 0:
                            for h2 in range(2):
                                o = h2 * QB
                                nc.gpsimd.affine_select(
                                    out=ae[:, o:o + P], in_=ae[:, o:o + P],
                                    compare_op=mybir.AluOpType.is_ge,
                                    fill=0.0, base=0,
                                    pattern=[[1, P]], channel_multiplier=-1)
                    pend.append((ki, d, ae, idx == 0, idx == n - 1))
                    if len(pend) > CFG.get("lag", 1):
                        _av(nc, vt, ph, ph2, g, pend.pop(0))
                while pend:
                    _av(nc, vt, ph, ph2, g, pend.pop(0))

                # normalize: rowsums at row 64 of ph / ph2
                rs = rsp.tile([P, 2 * QB], F32R, tag="rs", name="rs")
                nc.vector.tensor_copy(rs[D:D + 1, 0:QB], ph[D:D + 1, :])
                nc.vector.tensor_copy(rs[D:D + 1, QB:2 * QB], ph2[D:D + 1, :])
                rc = rcp.tile([D, 2 * QB], F32, tag="rc", name="rc")
                bc = bcp.tile([D, QB], F32, tag="bc", name="bc")
                nc.tensor.matmul(bc[:, :], onrt[D:D + 1, :],
                                 rs[D:D + 1, 0:QB], start=True, stop=True)
                nc.vector.reciprocal_approx_fast(rc[:, 0:QB], bc[:, :])
                bc2 = bcp.tile([D, QB], F32, tag="bc", name="bc")
                nc.tensor.matmul(bc2[:, :], onrt[D:D + 1, :],
                                 rs[D:D + 1, QB:2 * QB], start=True, stop=True)
                nc.vector.reciprocal_approx_fast(rc[:, QB:2 * QB], bc2[:, :])
                ytile = yt[g][qb]
                nc.vector.tensor_tensor(
                    ytile[0:D, :], ph[0:D, :], rc[0:D, 0:QB], MUL)
                nc.vector.tensor_tensor(
                    ytile[D:P, :], ph2[0:D, :], rc[0:D, QB:2 * QB], MUL)

            # out projection: out[t, m] = sum_i yT[i, t] * wpT[i, m]
            for tt in range(T // P):
                qb = tt // 4
                tl = (tt % 4) * P
                for mh in range(2):
                    if CFG.get("proj_mix") and (tt * 2 + mh) % 2 == 1:
                        po = avp.tile([P, QB], F32, tag="ph", name="po")
                    else:
                        po_full = pop.tile([P, 2 * QB], F32, tag="att",
                                           name="po")
                        po = po_full[:, 0:QB]
                    for g in range(4):
                        nc.tensor.matmul(
                            po[:], yt[g][qb][:, tl:tl + P],
                            wpt[:, g, mh * QB:(mh + 1) * QB],
                            start=(g == 0), stop=(g == 3))
                    ob = outp.tile([P, QB], F32, tag="ob", name="ob")
                    nc.vector.tensor_copy(ob[:], po[:])
                    nc.sync.dma_start(
                        out[tt * P:(tt + 1) * P, mh * QB:(mh + 1) * QB],
                        ob[:])

    nc.finalize()
    return nc


def _av(nc, vt, ph, ph2, g, job):
    """Emit the two AV matmuls (col-packed [v|ones], M=65) for one k-tile."""
    ki, d, ae, first, last = job
    if d == QB - P:
        d = QB - 2 * P  # [QB-2P:QB-P) zeroed above; N=256 runs at 1 cyc/row
    for h2 in range(2):
        h = 2 * g + h2
        o = h2 * QB
        psum_t = ph if h2 == 0 else ph2
        lh = vt[ki][:, h, 0:W65]
        if d > 0:
            nc.tensor.matmul(psum_t[0:W65, d:], lh, ae[:, o + d:o + QB],
                             start=first, stop=last)
        else:
            nc.tensor.matmul(psum_t[0:W65, :], lh, ae[:, o:o + QB],
                             start=first, stop=last)


def _prep_inputs(x, Wq, Wk, Wv, Wp):
    import math
    scale = 1.0 / math.sqrt(D)
    f32 = np.float32
    ones8 = np.ones((P, CO), f32)
    onr = np.ones((P, D), f32)
    zr = np.zeros((P, P), f32)
    in_maps = []
    for c in range(NC):
        b, g2 = c // 2, c % 2
        j0 = g2 * 512
        in_maps.append({
            "xt": np.ascontiguousarray(x[b].T.astype(f32)),
            "wq": np.ascontiguousarray((Wq[j0:j0 + 512] * scale).T.astype(f32)),
            "wk": np.ascontiguousarray(Wk[j0:j0 + 512].T.astype(f32)),
            "wv": np.ascontiguousarray(Wv[j0:j0 + 512].T.astype(f32)),
            "wp": np.ascontiguousarray(Wp[:, j0:j0 + 512].T.astype(f32)),
            "ones8": ones8,
            "onr": onr,
            "zr": zr,
        })
    return in_maps


def kernel(x, Wq, Wk, Wv, Wp, _trace=False):
    from concourse.bass_utils import run_bass_kernel_spmd

    x = np.asarray(x); Wq = np.asarray(Wq); Wk = np.asarray(Wk)
    Wv = np.asarray(Wv); Wp = np.asarray(Wp)

    if "nc" not in _CACHE:
        _CACHE["nc"] = _build()
    nc = _CACHE["nc"]

    in_maps = _prep_inputs(x, Wq, Wk, Wv, Wp)
    res = run_bass_kernel_spmd(nc, in_maps, core_ids=list(range(NC)),
                               trace=_trace)
    outs = [r["out"] for r in res.results]
    full = np.empty((B, T, C), np.float32)
    for b in range(B):
        full[b] = outs[2 * b] + outs[2 * b + 1]
    if _trace:
        _CACHE["last_results"] = res
    return full



# revision 22
# speedup vs baseline: 1.4863x; 1.4863x over previous
"""Causal self-attention (B=4, T=2048, C=1024, H=16) on 8 TRN2 NeuronCores.

Sharding: core c -> batch b = c//2, head-group g2 = c%2 (8 heads, feature
columns j0 = g2*512 .. +512).  Each core:
  - QKV projections for its 512-wide slice (Megatron column-parallel),
  - causal attention for its 8 heads (softmax without max-subtraction:
    logits ~ N(0,1), folded 1/sqrt(hd) into Wq on host),
  - partial output projection y_half @ Wp[:, slice].T.
Host sums the two partials per batch.  No collectives.

v3 dataflow (per core):
  - QKV projections in fp8e4 DoubleRow (2 k-tiles / matmul, 0.5 cyc/row)
    with first-order error compensation: x and W are split hi/lo into two
    fp8 tensors on the host (per-tensor power-of-2 scales keep the lo parts
    out of the fp8 subnormal range); psum accumulates xh*wh + xh*wl + xl*wh.
  - QK^T per k-tile in fp32r (exp amplifies logit error; fp8 not safe).
  - exp on ScalarE with scale=2^-13 (undoes the host scales), bf16 out.
  - causal staircase zeroed by gpsimd affine_select (bf16).
  - AV with the attention tile as the *stationary* operand [128k x 128q]
    and [v | ones]-moving (65 columns out): rowsum lands in psum column 64,
    so normalization is a per-partition reciprocal + one broadcast multiply
    on DVE (no broadcast matmuls).  AV psums accumulate with start=False
    onto gpsimd-memset banks (8 interleaved groups share 2 banks; the HW
    2KB zero-region would corrupt interleaved start=True groups).
  - y transposed back to [i, t] via PE transpose (bf16, 2 heads / transpose),
    then the output projection contracts in bf16.
  - fused software-pipelined schedule: the attention phase is ACT(exp)-
    limited, so next-t-block projections and previous-block transposes/
    out-projections are drained into the attention blocks as background PE
    work between k-tiles (the `Bg` queue of emission generators).
"""
import numpy as np

B, T, C = 4, 2048, 1024
NC = 8
P = 128
CO = 8           # c-tiles of 128 (contraction for QKV)
NCP = 4          # co-pairs (DoubleRow contracts 2 c-tiles per matmul)
QB = 512         # t_q block
NQB = T // QB    # 4
NKT = T // P     # 16 k-tiles
D = 64           # head dim
W65 = 65         # [v | ones]

S_Q = 256.0      # host scale on Wq/8 (fp8 dynamic range)
S_K = 32.0       # host scale on Wk
S_V = 32.0       # host scale on Wv (cancels in normalization via ones=S_V)
EXP_SCALE = 1.0 / (S_Q * S_K)   # 2^-13, applied inside the exp activation

_CACHE = {}

# build-time tuning knobs (sweepable)
CFG = {"attp": 2, "aep": 8, "lag": 4, "quantum": 1024}


class Bg:
    """Queue of emission generators drained between attention k-tiles.

    Each generator yields the matmul-row count it just emitted; items are
    labeled so attention blocks can force-drain their dependencies.
    """

    def __init__(self):
        self.items = []

    def add(self, label, gen):
        self.items.append((label, gen))

    def rows_left(self):
        return sum(1 for _ in self.items)  # item count proxy (unused)

    def drain_rows(self, target):
        done = 0
        while self.items and done < target:
            _, g = self.items[0]
            try:
                done += next(g)
            except StopIteration:
                self.items.pop(0)
        return done

    def drain_until(self, label):
        while any(l == label for l, _ in self.items):
            _, g = self.items[0]
            try:
                next(g)
            except StopIteration:
                self.items.pop(0)

    def drain_all(self):
        while self.items:
            _, g = self.items[0]
            try:
                next(g)
            except StopIteration:
                self.items.pop(0)


def _build():
    from contextlib import ExitStack
    import concourse.bass as bass
    import concourse.tile as tile
    from concourse import bacc, mybir

    F32 = mybir.dt.float32
    F32R = mybir.dt.float32r
    F8 = mybir.dt.float8e4
    BF16 = mybir.dt.bfloat16
    AF = mybir.ActivationFunctionType
    MUL = mybir.AluOpType.mult
    DR = mybir.MatmulPerfMode.DoubleRow

    nc = bacc.Bacc("TRN2", target_bir_lowering=False, debug=False,
                   dynamic_dma_scratch_size=2048)
    xh = nc.dram_tensor("xh", [C, T], F8, kind="ExternalInput").ap()
    xl = nc.dram_tensor("xl", [C, T], F8, kind="ExternalInput").ap()
    wts_d = {}
    for nm in ("wqh", "wql", "wkh", "wkl", "wvh", "wvl"):
        wts_d[nm] = nc.dram_tensor(nm, [C, 512], F8, kind="ExternalInput").ap()
    wp = nc.dram_tensor("wp", [512, C], BF16, kind="ExternalInput").ap()
    on1 = nc.dram_tensor("on1", [P, CO], BF16, kind="ExternalInput").ap()
    idt = nc.dram_tensor("idt", [P, P], BF16, kind="ExternalInput").ap()
    out = nc.dram_tensor("out", [T, C], F32, kind="ExternalOutput").ap()

    xh3 = xh.rearrange("(co ci) t -> ci co t", ci=P)     # [128, 8, 2048]
    xl3 = xl.rearrange("(co ci) t -> ci co t", ci=P)
    w3 = {nm: a.rearrange("(co ci) j -> ci co j", ci=P)
          for nm, a in wts_d.items()}                    # [128, 8, 512]
    wp3 = wp.rearrange("(go gi) m -> gi go m", gi=P)     # [128, 4, 1024]

    with tile.TileContext(nc) as tc, ExitStack() as ctx:
        persist = ctx.enter_context(tc.tile_pool(name="persist", bufs=1))
        qt = [persist.tile([P, T], F32R, tag=f"qt{g}", name=f"qt{g}") for g in range(4)]
        kt = [persist.tile([P, T], F32R, tag=f"kt{g}", name=f"kt{g}") for g in range(4)]
        vtp = persist.tile([P, NKT, CO, W65], BF16, tag="vtp", name="vtp")
        # normalized y, qtile-major: [q-pos, qtile, h2, d] (contiguous
        # [128,128] per-qtile slice for the PE transpose)
        ynm = [persist.tile([P, NKT, 2, D], BF16, tag=f"ynm{g}", name=f"ynm{g}")
               for g in range(4)]
        yts = [persist.tile([P, T], BF16, tag=f"yts{g}", name=f"yts{g}")
               for g in range(4)]
        on1t = persist.tile([P, CO], BF16, tag="on1", name="on1")
        idtt = persist.tile([P, P], BF16, tag="idt", name="idt")
        wpt = persist.tile([P, 4, C], BF16, tag="wpt", name="wpt")

        xtp = ctx.enter_context(tc.tile_pool(name="xtp", bufs=2))
        wpool = ctx.enter_context(tc.tile_pool(name="wqkv", bufs=1))
        bgp = ctx.enter_context(
            tc.tile_pool(name="bgp", bufs=2, space="PSUM"))
        attp = ctx.enter_context(
            tc.tile_pool(name="attp", bufs=CFG["attp"], space="PSUM"))
        avp = ctx.enter_context(tc.tile_pool(name="avp", bufs=1, space="PSUM"))
        aep = ctx.enter_context(tc.tile_pool(name="aep", bufs=CFG["aep"]))
        rcp = ctx.enter_context(tc.tile_pool(name="rcp", bufs=2))
        outp = ctx.enter_context(tc.tile_pool(name="outp", bufs=4))

        wt = {}
        for nm in ("wqh", "wql", "wkh", "wkl", "wvh", "wvl"):
            wt[nm] = wpool.tile([P, CO, 512], F8, tag=nm, name=nm)

        # ---- input DMAs: hi parts on sync, lo parts on vector (parallel
        # queues halve the head's arrival ramp); wk on gpsimd, v/wp on scalar
        xts = {}
        xh0 = xtp.tile([P, CO, QB], F8, tag="xh", name="xh0")
        xl0 = xtp.tile([P, CO, QB], F8, tag="xl", name="xl0")
        xts[0] = (xh0, xl0)
        for cp in range(NCP):
            s = slice(2 * cp, 2 * cp + 2)
            nc.sync.dma_start(wt["wqh"][:, s], w3["wqh"][:, s])
            nc.sync.dma_start(xh0[:, s], xh3[:, s, 0:QB])
            nc.gpsimd.dma_start(wt["wql"][:, s], w3["wql"][:, s])
            nc.gpsimd.dma_start(xl0[:, s], xl3[:, s, 0:QB])
        nc.scalar.dma_start(wt["wkh"][:], w3["wkh"])
        nc.scalar.dma_start(wt["wkl"][:], w3["wkl"])
        nc.scalar.dma_start(on1t[:], on1)
        nc.scalar.dma_start(idtt[:], idt)
        nc.scalar.dma_start(wt["wvh"][:], w3["wvh"])
        nc.scalar.dma_start(wt["wvl"][:], w3["wvl"])
        nc.scalar.dma_start(wpt[:], wp3)
        # rowsum column of v: ones * S_V (gpsimd; DVE is busy with psum moves)
        nc.gpsimd.tensor_copy(
            vtp[:, :, :, D:W65],
            on1t[:, None, :, None].broadcast_to((P, NKT, CO, 1)))

        # ---------- emission generators ----------
        def gen_qk(proj, g, tb):
            """q/k projection for one 128-wide j-slice, one 512-t block."""
            wh, wl = wt[f"w{proj}h"], wt[f"w{proj}l"]
            xh_t, xl_t = xts[tb]
            terms = ((xh_t, wh), (xh_t, wl), (xl_t, wh))
            dst = (qt if proj == "q" else kt)[g]
            gs = slice(g * P, (g + 1) * P)
            ps = bgp.tile([P, QB], F32, tag="bg", name=f"{proj}{g}t{tb}")
            # halves sequential: a start=True re-arms the bank zero-region
            for h in range(2):
                hs = slice(h * 256, (h + 1) * 256)
                for cp in range(NCP):
                    s = slice(2 * cp, 2 * cp + 2)
                    for ti, (mv, st) in enumerate(terms):
                        nc.tensor.matmul(
                            ps[:, hs], st[:, s, gs], mv[:, s, hs],
                            start=(cp == 0 and ti == 0),
                            stop=(cp == NCP - 1 and ti == 2),
                            perf_mode=DR)
                        yield 128
            nc.vector.tensor_copy(dst[:, tb * QB:(tb + 1) * QB], ps[:])
            yield 0

        def gen_v(tt, tb):
            """v projection (natural layout) for one 128-t tile."""
            wh, wl = wt["wvh"], wt["wvl"]
            xh_t, xl_t = xts[tb]
            terms = ((xh_t, wh), (xh_t, wl), (xl_t, wh))
            ki = tb * 4 + tt
            ts_ = slice(tt * P, (tt + 1) * P)
            ps = bgp.tile([P, QB], F32, tag="bg", name=f"v{ki}")
            for h in range(2):
                hs = slice(h * 256, (h + 1) * 256)
                for cp in range(NCP):
                    s = slice(2 * cp, 2 * cp + 2)
                    for ti, (mv, st) in enumerate(terms):
                        nc.tensor.matmul(
                            ps[:, hs], mv[:, s, ts_], st[:, s, hs],
                            start=(cp == 0 and ti == 0),
                            stop=(cp == NCP - 1 and ti == 2),
                            perf_mode=DR)
                        yield 128
            nc.vector.tensor_copy(
                vtp[:, ki, :, 0:D],
                ps[:].rearrange("p (h d) -> p h d", d=D))
            yield 0

        def gen_tr(g, qb):
            """transpose y_norm -> yT for one head-pair, one 512-t block."""
            psf = bgp.tile([P, QB], F32, tag="bg", name=f"tr{g}q{qb}")
            tp = psf[:].bitcast(BF16)[:, 0:QB].rearrange(
                "p (a b) -> p a b", a=4)
            for qt_ in range(4):
                nc.tensor.matmul(
                    tp[:, qt_, :],
                    ynm[g][:, qb * 4 + qt_, :, :].rearrange("p a b -> p (a b)"),
                    idtt[:], is_transpose=True)
                yield 128
            nc.vector.tensor_copy(
                yts[g][:, qb * QB:(qb + 1) * QB].rearrange(
                    "p (a b) -> p a b", a=4),
                tp[:])
            yield 0

        def gen_po(tt, mh, copy_on_scalar=False):
            """output projection for one [128 t, 512 m] tile + store."""
            po = bgp.tile([P, QB], F32, tag="bg", name=f"po{tt}m{mh}")
            for g in range(4):
                nc.tensor.matmul(
                    po[:], yts[g][:, tt * P:(tt + 1) * P],
                    wpt[:, g, mh * QB:(mh + 1) * QB],
                    start=(g == 0), stop=(g == 3))
                yield 512
            ob = outp.tile([P, QB], F32, tag="ob", name="ob")
            if copy_on_scalar:
                nc.scalar.activation(ob[:], po[:], AF.Copy)
            else:
                nc.vector.tensor_copy(ob[:], po[:])
            nc.sync.dma_start(
                out[tt * P:(tt + 1) * P, mh * QB:(mh + 1) * QB], ob[:])
            yield 0

        def run(gen):
            for _ in gen:
                pass

        # ---------- attention block ----------
        def att_block(g, qb, bg, pre_av=None):
            q0 = qb * QB
            ks = list(range(qb * 4, qb * 4 + 4)) + list(range(0, qb * 4))
            yp = avp.tile([P, 4, 2, P], F32, tag="yp", name="yp")
            nc.vector.memset(yp[:, :, :, 0:W65], 0.0)
            barrier = [pre_av]

            def av(job):
                if barrier[0] is not None:
                    barrier[0]()
                    barrier[0] = None
                ki, d, ae = job
                qt0 = 0 if d < 0 else d // P
                for h2 in range(2):
                    h = 2 * g + h2
                    for qt_ in range(qt0, 4):
                        nc.tensor.matmul(
                            yp[:, qt_, h2, 0:W65],
                            ae[:, h2, qt_ * P:(qt_ + 1) * P],
                            vtp[:, ki, h, 0:W65],
                            start=False, stop=True, skip_group_check=True)

            pend = []
            for idx, ki in enumerate(ks):
                d = (ki - qb * 4) * P if ki >= qb * 4 else -1
                dq = d if d in (P, 2 * P) else (2 * P if d == 3 * P else 0)
                ap_t = attp.tile([P, 2, QB], F32, tag="att", name="att")
                for h2 in range(2):
                    rows = slice(h2 * D, h2 * D + D)
                    nc.tensor.matmul(
                        ap_t[:, h2, dq:QB],
                        kt[g][rows, ki * P:(ki + 1) * P],
                        qt[g][rows, q0 + dq:q0 + QB],
                        start=True, stop=True)
                ae = aep.tile([P, 2, QB], BF16, tag="ae", name="ae")
                e0 = max(d, 0)
                nc.scalar.activation(ae[:, :, e0:QB], ap_t[:, :, e0:QB],
                                     AF.Exp, scale=EXP_SCALE)
                if d >= 0:
                    for h2 in range(2):
                        nc.gpsimd.affine_select(
                            out=ae[:, h2, d:d + P],
                            in_=ae[:, h2, d:d + P],
                            compare_op=mybir.AluOpType.is_ge,
                            fill=0.0, base=0,
                            pattern=[[1, P]], channel_multiplier=-1)
                pend.append((ki, d, ae))
                if len(pend) > CFG["lag"]:
                    av(pend.pop(0))
                bg.drain_rows(CFG["quantum"])
            while pend:
                av(pend.pop(0))

            rc = rcp.tile([P, 4, 2], F32, tag="rc", name="rc")
            nc.vector.reciprocal_approx_fast(rc[:], yp[:, :, :, D])
            nc.vector.tensor_tensor(
                ynm[g][:, qb * 4:(qb + 1) * 4, :, :],
                yp[:, :, :, 0:D],
                rc[:, :, :, None].broadcast_to((P, 4, 2, D)), MUL)

        # ---------- fused schedule ----------
        bg = Bg()
        # head: only q/k for head-pair 0; v follows in bg (first-AV barrier)
        run(gen_qk("q", 0, 0))
        run(gen_qk("k", 0, 0))
        for tt in range(4):
            bg.add("v0", gen_v(tt, 0))
        for g in range(1, 4):
            bg.add(f"qk{g}t0", gen_qk("q", g, 0))
            bg.add(f"qk{g}t0", gen_qk("k", g, 0))

        for qb in range(NQB):
            tbn = qb + 1
            if tbn < NQB:
                xh_t = xtp.tile([P, CO, QB], F8, tag="xh", name=f"xh{tbn}")
                xl_t = xtp.tile([P, CO, QB], F8, tag="xl", name=f"xl{tbn}")
                nc.sync.dma_start(xh_t[:], xh3[:, :, tbn * QB:(tbn + 1) * QB])
                nc.gpsimd.dma_start(xl_t[:], xl3[:, :, tbn * QB:(tbn + 1) * QB])
                xts[tbn] = (xh_t, xl_t)
                for tt in range(4):
                    bg.add(f"v{tbn}", gen_v(tt, tbn))
                for g in range(2):
                    bg.add(f"qk{g}t{tbn}", gen_qk("q", g, tbn))
                    bg.add(f"qk{g}t{tbn}", gen_qk("k", g, tbn))
            if qb > 0:
                for g in range(4):
                    bg.add(f"tr{qb - 1}", gen_tr(g, qb - 1))
                for tt in range(4 * (qb - 1), 4 * qb):
                    for mh in range(2):
                        bg.add(f"po{qb - 1}", gen_po(tt, mh))
            if tbn < NQB:
                # late j-slices feed the back half of this qb / early next qb
                for g in range(2, 4):
                    bg.add(f"qk{g}t{tbn}", gen_qk("q", g, tbn))
                    bg.add(f"qk{g}t{tbn}", gen_qk("k", g, tbn))
            for g in range(4):
                bg.drain_until(f"qk{g}t{qb}")
                pre = (lambda q_=qb: bg.drain_until(f"v{q_}")) if g == 0 else None
                att_block(g, qb, bg, pre_av=pre)
                if qb == NQB - 1:
                    bg.add("tr3", gen_tr(g, qb))

        bg.drain_all()
        for tt in range(4 * (NQB - 1), 4 * NQB):
            for mh in range(2):
                run(gen_po(tt, mh, copy_on_scalar=True))

    nc.finalize()
    return nc


def _prep_inputs(x, Wq, Wk, Wv, Wp):
    import ml_dtypes
    F8 = ml_dtypes.float8_e4m3
    BF = ml_dtypes.bfloat16
    f32 = np.float32

    def hilo(a):
        hi = np.ascontiguousarray(a).astype(F8)
        lo = (a - hi.astype(f32)).astype(F8)
        return hi, lo

    on1 = np.full((P, CO), S_V, BF)
    idt = np.eye(P, dtype=BF)
    in_maps = []
    for c in range(NC):
        b, g2 = c // 2, c % 2
        j0 = g2 * 512
        xhc, xlc = hilo(x[b].T.astype(f32))
        wqh, wql = hilo((Wq[j0:j0 + 512] * (S_Q / 8.0)).T.astype(f32))
        wkh, wkl = hilo((Wk[j0:j0 + 512] * S_K).T.astype(f32))
        wvh, wvl = hilo((Wv[j0:j0 + 512] * S_V).T.astype(f32))
        in_maps.append({
            "xh": xhc, "xl": xlc,
            "wqh": wqh, "wql": wql,
            "wkh": wkh, "wkl": wkl,
            "wvh": wvh, "wvl": wvl,
            "wp": np.ascontiguousarray(Wp[:, j0:j0 + 512].T).astype(BF),
            "on1": on1, "idt": idt,
        })
    return in_maps


def kernel(x, Wq, Wk, Wv, Wp, _trace=False):
    from concourse.bass_utils import run_bass_kernel_spmd

    x = np.asarray(x); Wq = np.asarray(Wq); Wk = np.asarray(Wk)
    Wv = np.asarray(Wv); Wp = np.asarray(Wp)

    if "nc" not in _CACHE:
        _CACHE["nc"] = _build()
    nc = _CACHE["nc"]

    in_maps = _prep_inputs(x, Wq, Wk, Wv, Wp)
    res = run_bass_kernel_spmd(nc, in_maps, core_ids=list(range(NC)),
                               trace=_trace)
    outs = [r["out"] for r in res.results]
    full = np.empty((B, T, C), np.float32)
    for b in range(B):
        full[b] = outs[2 * b] + outs[2 * b + 1]
    if _trace:
        _CACHE["last_results"] = res
    return full


# revision 29
# speedup vs baseline: 1.4963x; 1.0067x over previous
"""Causal self-attention (B=4, T=2048, C=1024, H=16) on 8 TRN2 NeuronCores.

Sharding: core c -> batch b = c//2, head-group g2 = c%2 (8 heads, feature
columns j0 = g2*512 .. +512).  Each core:
  - QKV projections for its 512-wide slice (Megatron column-parallel),
  - causal attention for its 8 heads (softmax without max-subtraction:
    logits ~ N(0,1), folded 1/sqrt(hd) into Wq on host),
  - partial output projection y_half @ Wp[:, slice].T.
Host sums the two partials per batch.  No collectives.

v3 dataflow (per core):
  - QKV projections in fp8e4 DoubleRow (2 k-tiles / matmul, 0.5 cyc/row)
    with first-order error compensation: x and W are split hi/lo into two
    fp8 tensors on the host (per-tensor power-of-2 scales keep the lo parts
    out of the fp8 subnormal range); psum accumulates xh*wh + xh*wl + xl*wh.
  - QK^T per k-tile in fp32r (exp amplifies logit error; fp8 not safe).
  - exp on ScalarE with scale=2^-13 (undoes the host scales), bf16 out.
  - causal staircase zeroed by gpsimd affine_select (bf16).
  - AV with the attention tile as the *stationary* operand [128k x 128q]
    and [v | ones]-moving (65 columns out): rowsum lands in psum column 64,
    so normalization is a per-partition reciprocal + one broadcast multiply
    on DVE (no broadcast matmuls).  AV psums accumulate with start=False
    onto gpsimd-memset banks (8 interleaved groups share 2 banks; the HW
    2KB zero-region would corrupt interleaved start=True groups).
  - y transposed back to [i, t] via PE transpose (bf16, 2 heads / transpose),
    then the output projection contracts in bf16.
  - fused software-pipelined schedule: the attention phase is ACT(exp)-
    limited, so next-t-block projections and previous-block transposes/
    out-projections are drained into the attention blocks as background PE
    work between k-tiles (the `Bg` queue of emission generators).
"""
import numpy as np

B, T, C = 4, 2048, 1024
NC = 8
P = 128
CO = 8           # c-tiles of 128 (contraction for QKV)
NCP = 4          # co-pairs (DoubleRow contracts 2 c-tiles per matmul)
QB = 512         # t_q block
NQB = T // QB    # 4
NKT = T // P     # 16 k-tiles
D = 64           # head dim
W65 = 65         # [v | ones]

S_Q = 256.0      # host scale on Wq/8 (fp8 dynamic range)
S_K = 32.0       # host scale on Wk
S_V = 32.0       # host scale on Wv (cancels in normalization via ones=S_V)
EXP_SCALE = 1.0 / (S_Q * S_K)   # 2^-13, applied inside the exp activation

_CACHE = {}

# build-time tuning knobs (sweepable)
CFG = {"attp": 2, "aep": 10, "lag": 6, "quantum": 896}


class Bg:
    """Queue of emission generators drained between attention k-tiles.

    Each generator yields the matmul-row count it just emitted; items are
    labeled so attention blocks can force-drain their dependencies.
    """

    def __init__(self):
        self.items = []

    def add(self, label, gen):
        self.items.append((label, gen))

    def rows_left(self):
        return sum(1 for _ in self.items)  # item count proxy (unused)

    def drain_rows(self, target):
        done = 0
        while self.items and done < target:
            _, g = self.items[0]
            try:
                done += next(g)
            except StopIteration:
                self.items.pop(0)
        return done

    def drain_until(self, label):
        while any(l == label for l, _ in self.items):
            _, g = self.items[0]
            try:
                next(g)
            except StopIteration:
                self.items.pop(0)

    def drain_all(self):
        while self.items:
            _, g = self.items[0]
            try:
                next(g)
            except StopIteration:
                self.items.pop(0)


def _build():
    from contextlib import ExitStack
    import concourse.bass as bass
    import concourse.tile as tile
    from concourse import bacc, mybir

    F32 = mybir.dt.float32
    F32R = mybir.dt.float32r
    F8 = mybir.dt.float8e4
    BF16 = mybir.dt.bfloat16
    AF = mybir.ActivationFunctionType
    MUL = mybir.AluOpType.mult
    DR = mybir.MatmulPerfMode.DoubleRow

    nc = bacc.Bacc("TRN2", target_bir_lowering=False, debug=False,
                   dynamic_dma_scratch_size=2048)
    xh = nc.dram_tensor("xh", [C, T], F8, kind="ExternalInput").ap()
    xl = nc.dram_tensor("xl", [C, T], F8, kind="ExternalInput").ap()
    wts_d = {}
    for nm in ("wqh", "wql", "wkh", "wkl", "wvh", "wvl"):
        wts_d[nm] = nc.dram_tensor(nm, [C, 512], F8, kind="ExternalInput").ap()
    wp = nc.dram_tensor("wp", [512, C], BF16, kind="ExternalInput").ap()
    on1 = nc.dram_tensor("on1", [P, CO], BF16, kind="ExternalInput").ap()
    idt = nc.dram_tensor("idt", [P, P], BF16, kind="ExternalInput").ap()
    out = nc.dram_tensor("out", [T, C], F32, kind="ExternalOutput").ap()

    xh3 = xh.rearrange("(co ci) t -> ci co t", ci=P)     # [128, 8, 2048]
    xl3 = xl.rearrange("(co ci) t -> ci co t", ci=P)
    w3 = {nm: a.rearrange("(co ci) j -> ci co j", ci=P)
          for nm, a in wts_d.items()}                    # [128, 8, 512]
    wp3 = wp.rearrange("(go gi) m -> gi go m", gi=P)     # [128, 4, 1024]

    with tile.TileContext(nc) as tc, ExitStack() as ctx:
        persist = ctx.enter_context(tc.tile_pool(name="persist", bufs=1))
        qt = [persist.tile([P, T], F32R, tag=f"qt{g}", name=f"qt{g}") for g in range(4)]
        kt = [persist.tile([P, T], F32R, tag=f"kt{g}", name=f"kt{g}") for g in range(4)]
        vtp = persist.tile([P, NKT, CO, W65], BF16, tag="vtp", name="vtp")
        # normalized y, qtile-major: [q-pos, qtile, h2, d] (contiguous
        # [128,128] per-qtile slice for the PE transpose)
        ynm = [persist.tile([P, NKT, 2, D], BF16, tag=f"ynm{g}", name=f"ynm{g}")
               for g in range(4)]
        yts = [persist.tile([P, T], BF16, tag=f"yts{g}", name=f"yts{g}")
               for g in range(4)]
        on1t = persist.tile([P, CO], BF16, tag="on1", name="on1")
        idtt = persist.tile([P, P], BF16, tag="idt", name="idt")
        wpt = persist.tile([P, 4, C], BF16, tag="wpt", name="wpt")

        xtp = ctx.enter_context(tc.tile_pool(name="xtp", bufs=2))
        wpool = ctx.enter_context(tc.tile_pool(name="wqkv", bufs=1))
        bgp = ctx.enter_context(
            tc.tile_pool(name="bgp", bufs=2, space="PSUM"))
        attp = ctx.enter_context(
            tc.tile_pool(name="attp", bufs=CFG["attp"], space="PSUM"))
        avp = ctx.enter_context(tc.tile_pool(name="avp", bufs=1, space="PSUM"))
        aep = ctx.enter_context(tc.tile_pool(name="aep", bufs=CFG["aep"]))
        rcp = ctx.enter_context(tc.tile_pool(name="rcp", bufs=2))
        outp = ctx.enter_context(tc.tile_pool(name="outp", bufs=4))

        wt = {}
        for nm in ("wqh", "wql", "wkh", "wkl", "wvh", "wvl"):
            wt[nm] = wpool.tile([P, CO, 512], F8, tag=nm, name=nm)

        # ---- input DMAs: hi parts on sync, lo parts on vector (parallel
        # queues halve the head's arrival ramp); wk on gpsimd, v/wp on scalar
        xts = {}
        xh0 = xtp.tile([P, CO, QB], F8, tag="xh", name="xh0")
        xl0 = xtp.tile([P, CO, QB], F8, tag="xl", name="xl0")
        xts[0] = (xh0, xl0)
        for cp in range(NCP):
            s = slice(2 * cp, 2 * cp + 2)
            nc.sync.dma_start(wt["wqh"][:, s], w3["wqh"][:, s])
            nc.sync.dma_start(xh0[:, s], xh3[:, s, 0:QB])
            nc.gpsimd.dma_start(wt["wql"][:, s], w3["wql"][:, s])
            nc.gpsimd.dma_start(xl0[:, s], xl3[:, s, 0:QB])
        nc.scalar.dma_start(wt["wkh"][:], w3["wkh"])
        nc.scalar.dma_start(wt["wkl"][:], w3["wkl"])
        nc.scalar.dma_start(on1t[:], on1)
        nc.scalar.dma_start(idtt[:], idt)
        nc.scalar.dma_start(wt["wvh"][:], w3["wvh"])
        nc.scalar.dma_start(wt["wvl"][:], w3["wvl"])
        nc.scalar.dma_start(wpt[:], wp3)
        # rowsum column of v: ones * S_V (gpsimd; DVE is busy with psum moves)
        nc.gpsimd.tensor_copy(
            vtp[:, :, :, D:W65],
            on1t[:, None, :, None].broadcast_to((P, NKT, CO, 1)))

        # ---------- emission generators ----------
        def gen_qk(proj, g, tb):
            """q/k projection for one 128-wide j-slice, one 512-t block."""
            wh, wl = wt[f"w{proj}h"], wt[f"w{proj}l"]
            xh_t, xl_t = xts[tb]
            terms = ((xh_t, wh), (xh_t, wl), (xl_t, wh))
            dst = (qt if proj == "q" else kt)[g]
            gs = slice(g * P, (g + 1) * P)
            ps = bgp.tile([P, QB], F32, tag="bg", name=f"{proj}{g}t{tb}")
            # halves sequential: a start=True re-arms the bank zero-region
            for h in range(2):
                hs = slice(h * 256, (h + 1) * 256)
                for cp in range(NCP):
                    s = slice(2 * cp, 2 * cp + 2)
                    for ti, (mv, st) in enumerate(terms):
                        nc.tensor.matmul(
                            ps[:, hs], st[:, s, gs], mv[:, s, hs],
                            start=(cp == 0 and ti == 0),
                            stop=(cp == NCP - 1 and ti == 2),
                            perf_mode=DR)
                        yield 128
            nc.vector.tensor_copy(dst[:, tb * QB:(tb + 1) * QB], ps[:])
            yield 0

        def gen_v(tt, tb):
            """v projection (natural layout) for one 128-t tile."""
            wh, wl = wt["wvh"], wt["wvl"]
            xh_t, xl_t = xts[tb]
            terms = ((xh_t, wh), (xh_t, wl), (xl_t, wh))
            ki = tb * 4 + tt
            ts_ = slice(tt * P, (tt + 1) * P)
            ps = bgp.tile([P, QB], F32, tag="bg", name=f"v{ki}")
            for h in range(2):
                hs = slice(h * 256, (h + 1) * 256)
                for cp in range(NCP):
                    s = slice(2 * cp, 2 * cp + 2)
                    for ti, (mv, st) in enumerate(terms):
                        nc.tensor.matmul(
                            ps[:, hs], mv[:, s, ts_], st[:, s, hs],
                            start=(cp == 0 and ti == 0),
                            stop=(cp == NCP - 1 and ti == 2),
                            perf_mode=DR)
                        yield 128
            nc.vector.tensor_copy(
                vtp[:, ki, :, 0:D],
                ps[:].rearrange("p (h d) -> p h d", d=D))
            yield 0

        def gen_tr(g, qb):
            """transpose y_norm -> yT for one head-pair, one 512-t block."""
            psf = bgp.tile([P, QB], F32, tag="bg", name=f"tr{g}q{qb}")
            tp = psf[:].bitcast(BF16)[:, 0:QB].rearrange(
                "p (a b) -> p a b", a=4)
            for qt_ in range(4):
                nc.tensor.matmul(
                    tp[:, qt_, :],
                    ynm[g][:, qb * 4 + qt_, :, :].rearrange("p a b -> p (a b)"),
                    idtt[:], is_transpose=True)
                yield 128
            nc.vector.tensor_copy(
                yts[g][:, qb * QB:(qb + 1) * QB].rearrange(
                    "p (a b) -> p a b", a=4),
                tp[:])
            yield 0

        def gen_po(tt, mh, copy_on_scalar=False):
            """output projection for one [128 t, 512 m] tile + store."""
            po = bgp.tile([P, QB], F32, tag="bg", name=f"po{tt}m{mh}")
            for g in range(4):
                nc.tensor.matmul(
                    po[:], yts[g][:, tt * P:(tt + 1) * P],
                    wpt[:, g, mh * QB:(mh + 1) * QB],
                    start=(g == 0), stop=(g == 3))
                yield 512
            ob = outp.tile([P, QB], F32, tag="ob", name="ob")
            if copy_on_scalar:
                nc.scalar.activation(ob[:], po[:], AF.Copy)
            else:
                nc.vector.tensor_copy(ob[:], po[:])
            nc.sync.dma_start(
                out[tt * P:(tt + 1) * P, mh * QB:(mh + 1) * QB], ob[:])
            yield 0

        def run(gen):
            for _ in gen:
                pass

        # ---------- attention block ----------
        def att_block(g, qb, bg, pre_av=None):
            q0 = qb * QB
            ks = list(range(qb * 4, qb * 4 + 4)) + list(range(0, qb * 4))
            yp = avp.tile([P, 4, 2, P], F32, tag="yp", name="yp")
            nc.vector.memset(yp[:, :, :, 0:W65], 0.0)
            barrier = [pre_av]

            def av(job):
                if barrier[0] is not None:
                    barrier[0]()
                    barrier[0] = None
                ki, d, ae = job
                qt0 = 0 if d < 0 else d // P
                for h2 in range(2):
                    h = 2 * g + h2
                    for qt_ in range(qt0, 4):
                        nc.tensor.matmul(
                            yp[:, qt_, h2, 0:W65],
                            ae[:, h2, qt_ * P:(qt_ + 1) * P],
                            vtp[:, ki, h, 0:W65],
                            start=False, stop=True, skip_group_check=True)

            pend = []
            for idx, ki in enumerate(ks):
                d = (ki - qb * 4) * P if ki >= qb * 4 else -1
                dq = d if d in (P, 2 * P) else (2 * P if d == 3 * P else 0)
                ap_t = attp.tile([P, 2, QB], F32, tag="att", name="att")
                for h2 in range(2):
                    rows = slice(h2 * D, h2 * D + D)
                    nc.tensor.matmul(
                        ap_t[:, h2, dq:QB],
                        kt[g][rows, ki * P:(ki + 1) * P],
                        qt[g][rows, q0 + dq:q0 + QB],
                        start=True, stop=True)
                ae = aep.tile([P, 2, QB], BF16, tag="ae", name="ae")
                e0 = max(d, 0)
                nc.scalar.activation(ae[:, :, e0:QB], ap_t[:, :, e0:QB],
                                     AF.Exp, scale=EXP_SCALE)
                if d >= 0:
                    for h2 in range(2):
                        nc.gpsimd.affine_select(
                            out=ae[:, h2, d:d + P],
                            in_=ae[:, h2, d:d + P],
                            compare_op=mybir.AluOpType.is_ge,
                            fill=0.0, base=0,
                            pattern=[[1, P]], channel_multiplier=-1)
                pend.append((ki, d, ae))
                if len(pend) > CFG["lag"]:
                    av(pend.pop(0))
                bg.drain_rows(CFG["quantum"])
            while pend:
                av(pend.pop(0))

            rc = rcp.tile([P, 4, 2], F32, tag="rc", name="rc")
            nc.vector.reciprocal_approx_fast(rc[:], yp[:, :, :, D])
            nc.vector.tensor_tensor(
                ynm[g][:, qb * 4:(qb + 1) * 4, :, :],
                yp[:, :, :, 0:D],
                rc[:, :, :, None].broadcast_to((P, 4, 2, D)), MUL)

        # ---------- fused schedule ----------
        bg = Bg()
        # head: only q/k for head-pair 0; v follows in bg (first-AV barrier)
        run(gen_qk("q", 0, 0))
        run(gen_qk("k", 0, 0))
        for tt in range(4):
            bg.add("v0", gen_v(tt, 0))
        for g in range(1, 4):
            bg.add(f"qk{g}t0", gen_qk("q", g, 0))
            bg.add(f"qk{g}t0", gen_qk("k", g, 0))

        for qb in range(NQB):
            tbn = qb + 1
            if tbn < NQB:
                xh_t = xtp.tile([P, CO, QB], F8, tag="xh", name=f"xh{tbn}")
                xl_t = xtp.tile([P, CO, QB], F8, tag="xl", name=f"xl{tbn}")
                nc.sync.dma_start(xh_t[:], xh3[:, :, tbn * QB:(tbn + 1) * QB])
                nc.gpsimd.dma_start(xl_t[:], xl3[:, :, tbn * QB:(tbn + 1) * QB])
                xts[tbn] = (xh_t, xl_t)
                for tt in range(4):
                    bg.add(f"v{tbn}", gen_v(tt, tbn))
                for g in range(2):
                    bg.add(f"qk{g}t{tbn}", gen_qk("q", g, tbn))
                    bg.add(f"qk{g}t{tbn}", gen_qk("k", g, tbn))
            if qb > 0:
                for g in range(4):
                    bg.add(f"tr{qb - 1}", gen_tr(g, qb - 1))
                for tt in range(4 * (qb - 1), 4 * qb):
                    for mh in range(2):
                        bg.add(f"po{qb - 1}", gen_po(tt, mh))
            if tbn < NQB:
                # late j-slices feed the back half of this qb / early next qb
                for g in range(2, 4):
                    bg.add(f"qk{g}t{tbn}", gen_qk("q", g, tbn))
                    bg.add(f"qk{g}t{tbn}", gen_qk("k", g, tbn))
            for g in range(4):
                bg.drain_until(f"qk{g}t{qb}")
                pre = (lambda q_=qb: bg.drain_until(f"v{q_}")) if g == 0 else None
                att_block(g, qb, bg, pre_av=pre)
                if qb == NQB - 1:
                    bg.add("tr3", gen_tr(g, qb))

        bg.drain_all()
        for tt in range(4 * (NQB - 1), 4 * NQB):
            for mh in range(2):
                run(gen_po(tt, mh, copy_on_scalar=True))

    nc.finalize()
    return nc


def _prep_inputs(x, Wq, Wk, Wv, Wp):
    import ml_dtypes
    F8 = ml_dtypes.float8_e4m3
    BF = ml_dtypes.bfloat16
    f32 = np.float32

    def hilo(a):
        hi = np.ascontiguousarray(a).astype(F8)
        lo = (a - hi.astype(f32)).astype(F8)
        return hi, lo

    on1 = np.full((P, CO), S_V, BF)
    idt = np.eye(P, dtype=BF)
    in_maps = []
    for c in range(NC):
        b, g2 = c // 2, c % 2
        j0 = g2 * 512
        xhc, xlc = hilo(x[b].T.astype(f32))
        wqh, wql = hilo((Wq[j0:j0 + 512] * (S_Q / 8.0)).T.astype(f32))
        wkh, wkl = hilo((Wk[j0:j0 + 512] * S_K).T.astype(f32))
        wvh, wvl = hilo((Wv[j0:j0 + 512] * S_V).T.astype(f32))
        in_maps.append({
            "xh": xhc, "xl": xlc,
            "wqh": wqh, "wql": wql,
            "wkh": wkh, "wkl": wkl,
            "wvh": wvh, "wvl": wvl,
            "wp": np.ascontiguousarray(Wp[:, j0:j0 + 512].T).astype(BF),
            "on1": on1, "idt": idt,
        })
    return in_maps


def kernel(x, Wq, Wk, Wv, Wp, _trace=False):
    from concourse.bass_utils import run_bass_kernel_spmd

    x = np.asarray(x); Wq = np.asarray(Wq); Wk = np.asarray(Wk)
    Wv = np.asarray(Wv); Wp = np.asarray(Wp)

    if "nc" not in _CACHE:
        _CACHE["nc"] = _build()
    nc = _CACHE["nc"]

    in_maps = _prep_inputs(x, Wq, Wk, Wv, Wp)
    res = run_bass_kernel_spmd(nc, in_maps, core_ids=list(range(NC)),
                               trace=_trace)
    outs = [r["out"] for r in res.results]
    full = np.empty((B, T, C), np.float32)
    for b in range(B):
        full[b] = outs[2 * b] + outs[2 * b + 1]
    if _trace:
        _CACHE["last_results"] = res
    return full
